# revision 31
# baseline (speedup 1.0000x reference)
"""CAM (channel attention) module kernel for Trainium2, 8 NeuronCores.

Reference computation (per batch b):
    q = x[b].reshape(C, N)                      # C=128, N=65536
    energy = q @ q.T                            # C x C
    att = softmax(rowmax(energy) - energy)      # == exp(rowmin(e)-e)/rowsum
    out = att @ q
    result = gamma * out + x

Sharding: cores 0-3 handle batch 0, cores 4-7 handle batch 1; each core
owns a contiguous N/4 = 16384 column slice.  Partial C x C energy is
AllReduced within each group of 4, softmax is computed redundantly
(tiny), and the AV matmul + residual are done on the local slice.

Numerics: the PE matmuls run fp16 with an hi/lo split for the energy
term:  q = qh + ql (fp16 each, ~22 mantissa bits combined), and
    E = Qh Qh^T + C + C^T,   C = sum_j Qh_j Ql_j^T
which keeps the absolute error of the 65536-length dot products small
enough for the softmax (exp) stage.  The hi/lo transposed chunks are
interleaved in one SBUF tensor so each energy step is a single N=256
matmul accumulating [E_hh | C] into one PSUM tile.  Transposes run on
the TensorE (xbar DMA transposes would serialize against the
collective).  The residual add uses the exact f32 copy of x.  gamma is
folded into the attention matrix, so the residual is a single add.
"""

import numpy as np

import concourse.bass as bass
import concourse.mybir as mybir
import concourse.tile as tile
from concourse import bacc
from concourse.bass_utils import run_bass_kernel_spmd
from concourse.masks import make_identity

B, C, D, H, W = 2, 128, 16, 64, 64
N = D * H * W  # 65536
NCORES = 8
SHARDS_PER_BATCH = 4
NS = N // SHARDS_PER_BATCH  # 16384 columns per core

F32 = mybir.dt.float32
F16 = mybir.dt.float16

# tuning knobs
CFG = dict(
    nb=1024,          # pipeline block (cast/sub granularity)
    load_plan=(512, 512, 1024, 2048, 4096, 4096, 4096),
    store_nb=2048,    # output store DMA granularity
    avf=512,          # AV matmul free-dim chunk
    av_bufs=3,
    use_collective=True,
    warmup_ar=True,
)


def _body(nc: bass.Bass, tc: "tile.TileContext", xs, gm, out, cfg):
    NB = cfg["nb"]
    AVF = cfg["avf"]
    JCH = NS // 128
    with (
        tc.tile_pool(name="big", bufs=1) as big,
        tc.tile_pool(name="small", bufs=1) as small,
        tc.tile_pool(name="work", bufs=4) as work,
        tc.tile_pool(name="qlb", bufs=3) as qlb,
        tc.tile_pool(name="psum_e", bufs=1, space="PSUM") as pse,
        tc.tile_pool(name="psum_av", bufs=cfg["av_bufs"], space="PSUM") as psav,
        tc.tile_pool(name="trps", bufs=2, space="PSUM") as trps,
        tc.tile_pool(name="dram", bufs=1, space="DRAM") as dram,
    ):
        # Persistent SBUF tensors
        xf = big.tile([C, NS], F32, tag="xf")        # exact f32 x (residual)
        qh = big.tile([C, NS], F16, tag="qh")        # fp16 hi, natural (AV rhs)
        # transposed chunks, [hi_j | lo_j] interleaved along the free dim
        qT = big.tile([128, JCH, 256], F16, tag="qT")

        identh = small.tile([128, 128], F16, tag="identh")
        make_identity(nc, identh)
        ident = small.tile([128, 128], F32, tag="ident")
        make_identity(nc, ident)

        g0 = small.tile([1, 1], F32, tag="g0")
        gsb = small.tile([128, 1], F32, tag="gsb")
        nc.sync.dma_start(g0[:], gm[None, :])
        nc.gpsimd.partition_broadcast(gsb, g0[:])

        if cfg["use_collective"] and cfg["warmup_ar"]:
            # Warm the collective path (comm init, CC firmware wakeup)
            # concurrently with phase 1 so the real AllReduce is cheap.
            w_in = dram.tile([1, 16], F32, tag="w_in")
            w_out = dram.tile([1, 16], F32, tag="w_out")
            w_sb = small.tile([1, 16], F32, tag="w_sb")
            nc.gpsimd.memset(w_sb, 0.0)
            nc.sync.dma_start(w_in[:], w_sb)
            nc.gpsimd.collective_compute(
                "AllReduce",
                mybir.AluOpType.add,
                replica_groups=[[0, 1, 2, 3], [4, 5, 6, 7]],
                ins=[w_in.opt()],
                outs=[w_out.opt()],
            )

        # ---- Stage 1: load -> split-cast -> PE-transpose -> energy MMs ----
        # ec_ps accumulates [E_hh | C] over all chunks
        ec_ps = pse.tile([128, 256], F32, tag="ec_ps")
        pos = 0
        for ln in cfg["load_plan"]:
            nc.sync.dma_start(xf[:, pos:pos + ln], xs[:, pos:pos + ln])
            pos += ln
        assert pos == NS

        GB = 512
        gjp = GB // 128   # 4 chunks per transpose group
        ngrp = NS // GB
        ngb = NB // GB or 1
        nblk = NS // NB

        def emit_emm(jlist):
            for j in jlist:
                nc.tensor.matmul(
                    ec_ps, lhsT=qT[:, j, 0:128], rhs=qT[:, j, :],
                    start=(j == 0), stop=(j == JCH - 1),
                )

        for blk in range(nblk):
            sl = slice(blk * NB, (blk + 1) * NB)
            nc.vector.tensor_copy(qh[:, sl], xf[:, sl])          # fp16 hi
            ql = qlb.tile([C, NB], F16, tag="ql")
            sub_eng = nc.vector if blk % 2 else nc.gpsimd
            sub_eng.tensor_tensor(                                # fp16 lo
                ql, xf[:, sl], qh[:, sl], mybir.AluOpType.subtract
            )
            for gg in range(NB // GB):
                g = blk * (NB // GB) + gg
                th = trps.tile([128, GB], F16, tag="th")
                tl = trps.tile([128, GB], F16, tag="tl")
                for u in range(gjp):
                    a0 = blk * NB + gg * GB + u * 128
                    r0 = gg * GB + u * 128
                    ps = slice(u * 128, (u + 1) * 128)
                    nc.tensor.transpose(th[:, ps], qh[:, a0:a0 + 128], identh)
                    nc.tensor.transpose(tl[:, ps], ql[:, r0:r0 + 128], identh)
                jsl = slice(g * gjp, (g + 1) * gjp)
                nc.scalar.copy(
                    qT[:, jsl, 0:128],
                    th.rearrange("p (a b) -> p a b", b=128),
                )
                nc.vector.tensor_copy(
                    qT[:, jsl, 128:256],
                    tl.rearrange("p (a b) -> p a b", b=128),
                )
                if g > 0:
                    emit_emm(range((g - 1) * gjp, g * gjp))
        emit_emm(range((ngrp - 1) * gjp, ngrp * gjp))

        # ---- Stage 2: E = E_hh + C + C^T, then AllReduce ----
        c_sb = small.tile([128, 128], F32, tag="c_sb")
        nc.vector.tensor_copy(c_sb, ec_ps[:, 128:256])
        cT_ps = trps.tile([128, 128], F32, tag="th")
        nc.tensor.transpose(cT_ps, c_sb, ident)
        e_sb = small.tile([128, 128], F32, tag="e_sb")
        nc.vector.tensor_add(e_sb, ec_ps[:, 0:128], c_sb)
        nc.vector.tensor_add(e_sb, e_sb, cT_ps)

        if cfg["use_collective"]:
            e_in = dram.tile([128, 128], F32, tag="e_in")
            e_out = dram.tile([128, 128], F32, tag="e_out")
            nc.sync.dma_start(e_in[:], e_sb)
            nc.gpsimd.collective_compute(
                "AllReduce",
                mybir.AluOpType.add,
                replica_groups=[[0, 1, 2, 3], [4, 5, 6, 7]],
                ins=[e_in.opt()],
                outs=[e_out.opt()],
            )
            e_full = small.tile([128, 128], F32, tag="e_full")
            nc.sync.dma_start(e_full, e_out[:])
        else:
            e_full = e_sb

        # ---- Stage 3: softmax (att = exp(rowmin(e) - e) / rowsum) ----
        m = small.tile([128, 1], F32, tag="m")
        nc.vector.tensor_reduce(
            m, e_full, axis=mybir.AxisListType.X, op=mybir.AluOpType.min
        )
        t = small.tile([128, 128], F32, tag="t")
        r = small.tile([128, 1], F32, tag="r")
        nc.scalar.activation(
            t, e_full, mybir.ActivationFunctionType.Exp,
            bias=m, scale=-1.0, accum_out=r,
        )
        rinv = small.tile([128, 1], F32, tag="rinv")
        nc.vector.reciprocal(rinv, r)
        gr = small.tile([128, 1], F32, tag="gr")
        nc.vector.tensor_mul(gr, rinv, gsb)
        att = small.tile([128, 128], F32, tag="att")
        nc.vector.tensor_scalar_mul(att, t, gr)   # att = gamma * softmax rows

        attT_ps = trps.tile([128, 128], F32, tag="th")
        nc.tensor.transpose(attT_ps, att, ident)
        attT = small.tile([128, 128], F16, tag="attT")
        nc.vector.tensor_copy(attT, attT_ps)

        # ---- Stage 4: AV matmul + residual + store ----
        SNB = cfg["store_nb"]
        per_store = SNB // AVF
        o_sb = None
        for f in range(NS // AVF):
            sl = slice(f * AVF, (f + 1) * AVF)
            av_ps = psav.tile([128, AVF], F32, tag="av_ps")
            nc.tensor.matmul(av_ps, lhsT=attT, rhs=qh[:, sl],
                             start=True, stop=True)
            if f % per_store == 0:
                o_sb = work.tile([128, SNB], F32, tag="o_sb")
            osl = slice((f % per_store) * AVF, (f % per_store + 1) * AVF)
            if f % 3 == 2:
                # alternate path: ACT evacuates PSUM (fp16), GpSimd adds —
                # keeps a third of the residual adds off the DVE
                avs = work.tile([128, AVF], F16, tag="avs")
                nc.scalar.copy(avs, av_ps)
                nc.gpsimd.tensor_add(o_sb[:, osl], avs, xf[:, sl])
            else:
                nc.vector.tensor_add(o_sb[:, osl], av_ps, xf[:, sl])
            if (f + 1) % per_store == 0:
                st = slice((f + 1 - per_store) * AVF, (f + 1) * AVF)
                dma_eng = nc.sync if (f // per_store) % 2 else nc.scalar
                dma_eng.dma_start(out[:, st], o_sb)


_cached_nc = None


def _build(cfg=None):
    cfg = dict(CFG, **(cfg or {}))
    nc = bacc.Bacc(
        "TRN2",
        target_bir_lowering=False,
        debug=False,
        enable_asserts=False,
        num_devices=NCORES,
    )
    xs = nc.dram_tensor("xs", [C, NS], F32, kind="ExternalInput").ap()
    gm = nc.dram_tensor("gamma", [1], F32, kind="ExternalInput").ap()
    out = nc.dram_tensor("out", [C, NS], F32, kind="ExternalOutput").ap()
    with tile.TileContext(nc) as tc:
        _body(nc, tc, xs, gm, out, cfg)
    nc.compile()
    return nc


def kernel(x: np.ndarray, gamma: np.ndarray, _collect_results=None) -> np.ndarray:
    global _cached_nc
    if _cached_nc is None:
        _cached_nc = _build()
    nc = _cached_nc

    xr = np.ascontiguousarray(np.asarray(x, dtype=np.float32).reshape(B, C, N))
    gamma = np.ascontiguousarray(np.asarray(gamma, dtype=np.float32))
    in_maps = []
    for k in range(NCORES):
        b, s = divmod(k, SHARDS_PER_BATCH)
        shard = np.ascontiguousarray(xr[b, :, s * NS:(s + 1) * NS])
        in_maps.append({"xs": shard, "gamma": gamma})

    res = run_bass_kernel_spmd(nc, in_maps, core_ids=list(range(NCORES)))
    if _collect_results is not None:
        _collect_results.append(res)

    outf = np.empty((B, C, N), np.float32)
    for k in range(NCORES):
        b, s = divmod(k, SHARDS_PER_BATCH)
        outf[b, :, s * NS:(s + 1) * NS] = res.results[k]["out"]
    return outf.reshape(B, C, D, H, W)
